# revision 3
# baseline (speedup 1.0000x reference)
"""Trainium2 Bass kernel for the NeRF renderer + distortion loss.

Strategy (per core, pure data-parallel over rays: 1024 rays / 8 cores = 128
rays = exactly the 128 SBUF partitions; samples N=576 live on the free dim):

  x      = sigma * delta                      (DVE tensor_tensor)
  x[...,-1] clamped to <= 87 (delta[-1]=1e10; exp(-87)+eps == eps, 1-exp(-87)==1
            in f32, so results are bit-identical to the unclamped math)
  t      = exp(-x)                            (ACT activation)
  q      = t + 1e-10                          (DVE tensor_scalar, 2x mode)
  trans  = exclusive-cumprod(q)               (DVE tensor_tensor_scan, mult)
  wneg   = (t - 1) * trans = -weights         (DVE scalar_tensor_tensor,
                                               accum_out -> -W = -sum w)
  wtn    = wneg * z_lin = -w*t                (DVE stt, accum_out -> -T)
  Pn     = inclusive-cumsum(wneg) = -P        (DVE tensor_tensor_scan, add)
  r1     = sum(wtn * Pn)  = sum(w*t * P)      (DVE tensor_tensor_reduce)
  r3     = sum(wneg * wtn) = sum(w^2 * t)     (DVE ttr)
  -img_c = sum(wneg * rgb_c), c=0,1,2         (DVE ttr; rgb channel-planar)
  -invd  = sum(wneg * 1/z)                    (DVE ttr)

Distortion loss per ray (z_lin sorted ascending):
  sum_ij w_i w_j |t_i - t_j| = 2*(2*sum(wt*P) - W*T - sum(w*w*t))
computed on host from the per-ray scalars; final mean over rays on host
(this is the "all-reduce" of the hint, done in the gather step).
"""

import sys

if "/opt/trn_rl_repo" not in sys.path:
    sys.path.insert(0, "/opt/trn_rl_repo")

import numpy as np

N_INNER, N_OUTER = 384, 192
N = N_INNER + N_OUTER          # 576 samples per ray
R = 1024                       # rays
NCORES = 8
RS = R // NCORES               # 128 rays per core == SBUF partitions
EPS = 1e-10
XCLAMP = 87.0                  # exp(-87) ~ 1.6e-38: +eps==eps, 1-it==1 in f32

_BUILT = None                  # cached compiled Bass module


def _constants():
    """[3, 576] f32: rows = (deltas, 1/z, z_lin). Matches reference's
    jnp.logspace/linspace (computed in f64, cast to f32; diffs are <=1ulp)."""
    zi = np.power(10.0, np.linspace(-1.2, 0.0, N_INNER)).astype(np.float32)
    zo = np.power(10.0, np.linspace(0.0, 2.0, N_OUTER)).astype(np.float32)
    z = np.concatenate([zi, zo]).astype(np.float32)
    zl = np.concatenate(
        [np.linspace(-1.2, 0.0, N_INNER), np.linspace(0.0, 2.0, N_OUTER)]
    ).astype(np.float32)
    zlin = ((zl + np.float32(1.2)) / np.float32(3.2)).astype(np.float32)
    deltas = np.concatenate([np.diff(z), np.array([1e10], np.float32)]).astype(
        np.float32
    )
    invz = (np.float32(1.0) / z).astype(np.float32)
    return np.stack([deltas, invz, zlin]).astype(np.float32)


def _build():
    import concourse.bass as bass
    import concourse.bacc as bacc
    import concourse.tile as tile
    from concourse import mybir

    Alu = mybir.AluOpType
    AF = mybir.ActivationFunctionType
    F32 = mybir.dt.float32

    nc = bacc.Bacc("TRN2", target_bir_lowering=False, debug=False)

    d_sig = nc.dram_tensor("sig", [RS, N], F32, kind="ExternalInput")
    d_rgb = nc.dram_tensor("rgb", [RS, 3 * N], F32, kind="ExternalInput")
    d_cst = nc.dram_tensor("cst", [3, N], F32, kind="ExternalInput")
    d_out = nc.dram_tensor("out", [RS, 8], F32, kind="ExternalOutput")

    with tile.TileContext(nc) as tc:
        with (
            tc.tile_pool(name="p", bufs=1) as p,
            tc.tile_pool(name="scr", bufs=2) as scrp,
        ):
            s_sig = p.tile([RS, N], F32)
            s_rgb = p.tile([RS, 3 * N], F32)
            s_delta = p.tile([RS, N], F32)
            s_invz = p.tile([RS, N], F32)
            s_zlin = p.tile([RS, N], F32)
            s_zero = p.tile([RS, N], F32)
            s_x = p.tile([RS, N], F32)
            s_t = p.tile([RS, N], F32)
            s_q = p.tile([RS, N], F32)
            s_transw = p.tile([RS, N + 1], F32)
            s_wneg = p.tile([RS, N], F32)
            s_wtn = p.tile([RS, N], F32)
            s_pn = p.tile([RS, N], F32)
            s_pack = p.tile([RS, 8], F32)

            # ---- loads ----
            nc.sync.dma_start(out=s_sig[:], in_=d_sig[:])
            nc.sync.dma_start(out=s_rgb[:], in_=d_rgb[:])
            for i, tl in enumerate([s_delta, s_invz, s_zlin]):
                row = d_cst[i : i + 1, :]
                bcast = bass.AP(
                    tensor=row.tensor, offset=row.offset, ap=[[0, RS], [1, N]]
                )
                nc.sync.dma_start(out=tl[:], in_=bcast)
            nc.gpsimd.memset(s_zero[:], 0.0)
            nc.gpsimd.memset(s_transw[:, 0:1], 1.0)

            # ---- compute ----
            nc.vector.tensor_mul(s_x[:], s_sig[:], s_delta[:])
            nc.vector.tensor_scalar_min(s_x[:, N - 1 : N], s_x[:, N - 1 : N], XCLAMP)
            nc.scalar.activation(s_t[:], s_x[:], AF.Exp, bias=0.0, scale=-1.0)
            nc.vector.tensor_scalar_add(s_q[:], s_t[:], EPS)
            nc.vector.tensor_tensor_scan(
                out=s_transw[:, 1 : N + 1],
                data0=s_q[:],
                data1=s_zero[:],
                initial=1.0,
                op0=Alu.mult,
                op1=Alu.add,
            )
            nc.vector.scalar_tensor_tensor(
                out=s_wneg[:],
                in0=s_t[:],
                scalar=1.0,
                in1=s_transw[:, 0:N],
                op0=Alu.subtract,
                op1=Alu.mult,
                accum_out=s_pack[:, 6:7],  # -W
            )
            nc.vector.scalar_tensor_tensor(
                out=s_wtn[:],
                in0=s_wneg[:],
                scalar=1.0,
                in1=s_zlin[:],
                op0=Alu.mult,
                op1=Alu.mult,
                accum_out=s_pack[:, 7:8],  # -T
            )
            nc.vector.tensor_tensor_scan(
                out=s_pn[:],
                data0=s_wneg[:],
                data1=s_zero[:],
                initial=0.0,
                op0=Alu.add,
                op1=Alu.add,
            )

            def ttr(in0, in1, col):
                # fused product+sum: out = (in0 * 1.0) * in1, accum = sum(out)
                # (tensor_tensor_reduce crashes the device; stt+accum works)
                s_scr = scrp.tile([RS, N], F32, tag="scr", name=f"scr{col}")
                nc.vector.scalar_tensor_tensor(
                    out=s_scr[:],
                    in0=in0,
                    scalar=1.0,
                    in1=in1,
                    op0=Alu.mult,
                    op1=Alu.mult,
                    accum_out=s_pack[:, col : col + 1],
                )

            ttr(s_wtn[:], s_pn[:], 4)        # r1 = sum(wt*P)
            ttr(s_wneg[:], s_wtn[:], 5)      # r3 = sum(w*w*t)
            for c in range(3):               # -img_c
                ttr(s_wneg[:], s_rgb[:, c * N : (c + 1) * N], c)
            ttr(s_wneg[:], s_invz[:], 3)     # -invdepth

            nc.sync.dma_start(out=d_out[:], in_=s_pack[:])

    nc.compile()
    return nc


def _run(sigmas, rgbs, trace=False):
    """Shard, run on 8 cores, gather. Returns ((image, invdepth, l_dist), ns)."""
    global _BUILT
    if _BUILT is None:
        _BUILT = _build()
    nc = _BUILT
    from concourse.bass_utils import run_bass_kernel_spmd

    sig = np.ascontiguousarray(np.asarray(sigmas, dtype=np.float32))
    rgb = np.asarray(rgbs, dtype=np.float32)
    # channel-planar per ray: [r, c*N + k]
    rgbp = np.ascontiguousarray(rgb.transpose(0, 2, 1)).reshape(R, 3 * N)
    cst = _constants()

    in_maps = [
        {
            "sig": sig[i * RS : (i + 1) * RS],
            "rgb": rgbp[i * RS : (i + 1) * RS],
            "cst": cst,
        }
        for i in range(NCORES)
    ]
    out = run_bass_kernel_spmd(nc, in_maps, list(range(NCORES)), trace=trace)
    res = np.concatenate(
        [out.results[i]["out"] for i in range(NCORES)], axis=0
    ).astype(np.float32)  # [1024, 8]

    image = (-res[:, 0:3]).astype(np.float32)      # [1024, 3]
    invdepth = (-res[:, 3]).astype(np.float32)     # [1024]
    r1, r3, nW, nT = res[:, 4], res[:, 5], res[:, 6], res[:, 7]
    lray = 2.0 * (2.0 * r1 - nW * nT - r3)
    l_dist = np.float32(np.mean(lray, dtype=np.float64))
    return (image[None], invdepth[None], l_dist), out.exec_time_ns


def kernel(sigmas, rgbs):
    (image, invdepth, l_dist), _ = _run(sigmas, rgbs, trace=False)
    return image, invdepth, l_dist
